# revision 6
# baseline (speedup 1.0000x reference)
"""Trainium2 Bass kernel for nn_DeepTimeGraphNet (per-row conv/pool pyramid + classifier).

Contract: kernel(**inputs) takes the FULL unsharded inputs (keys as in
setup_inputs()) and returns the FULL (64, 3) softmax output.

Sharding: pure data parallel over batch. Core i handles batch rows
[8i, 8i+8) = 8192 (batch, node) rows of length 1200. Inside each core the
rows are processed as 8 "supertiles" of 1024 rows = 128 SBUF partitions x 8
column groups, so every engine instruction covers 1024 rows at once.

v2 pipeline (weights baked in as immediates at trace time). Key facts from
the v1 trace: every DVE/ACT op's duration ~= bytes SPANNED per partition /
~8B/ns + ~180ns, the two DVE operand ports stream independently, and
strided writes with contiguous inner runs cost the same as dense ones.
Engine APs are limited to 2 free dims, which every op below respects via
stride-merged views. So:

- S1 (conv0 k2 s2) is ONE DVE scalar_tensor_tensor per half-supertile:
  y0' = x_even*(w_min/w_max) + x_odd (both stride-2 ports stream the same
  19.2KB window concurrently), with the conv0 scale kf=w_max and bias b0
  factored out, writing y0' de-interleaved into 3 maxpool phase planes
  (dst [(1600,3),(1,800)]; src [(2,3),(6,800)] since the group stride 1200
  = 6*200 merges with t). ScalarE's old S1 tap disappears entirely.
- S2 (maxpool3) = two contiguous tensor_tensor max/min ops on the planes
  (6.4KB span instead of 19.2KB). kf<0 flips max to min.
- S2b: ScalarE applies r1 = Relu(kf*m + b0) via two ACTIVATEs that write
  r1 de-interleaved into even/odd planes carrying built-in zero pads
  (E plane: data j=0..99 then pad; O plane: pad then data), so conv2's
  four taps are all full-range CONTIGUOUS reads.
- S3 (conv2 k4 s2 p1): 1 ACTIVATE + 3 stt, every tap a contiguous read
  from the padded planes, dst interleaved y2.
- S4 (maxpool2+relu): one stt max (stride-2 reads, contiguous dst r3).
- The Vec-side S3/S4 of supertile s are deferred to iteration s+1 so the
  ScalarE round trip (S2b+S3act) never stalls DVE; supertile 7 instead
  runs half-by-half immediately to shorten the post-DMA drain.
- Tail (conv4..conv8, 50->1) batched over supertile/group slices placed
  in DMA windows. Then PE matmul classifier + exact softmax.

The kernel is HBM-bound: 39.3 MB/core over 16 DMA engines x ~24.6 B/ns
~= 100us floor; v2 sizes DVE work (~9.3us/supertile) under the DMA rate
(~12.5us/supertile) so the stream packs.
"""
import os
import sys

for _p in ("/root/.axon_site/_ro/trn_rl_repo", "/opt/trn_rl_repo"):
    if os.path.isdir(_p) and _p not in sys.path:
        sys.path.insert(0, _p)

import numpy as np  # noqa: E402

import concourse.bacc as bacc  # noqa: E402
import concourse.tile as tile  # noqa: E402
from concourse import mybir  # noqa: E402
from concourse.ap import AP  # noqa: E402
from concourse.bass_utils import run_bass_kernel_spmd  # noqa: E402

F32 = mybir.dt.float32
Alu = mybir.AluOpType
Act = mybir.ActivationFunctionType

BS, NN, T = 64, 1024, 1200
N_CORES = 8
S_PER_CORE = 8          # supertiles per core; each = 1024 rows (one batch row)
C = 8                   # column groups per supertile (128 rows each)

_CACHE = {}


def _cap(base, off, dims):
    """Raw AP on base's tile: partition dim inherited, free dims given as
    (stride, size) pairs in iteration order (outer->inner)."""
    return AP(base.tensor, base.offset + off,
              [list(base.ap[0])] + [[st, sz] for st, sz in dims])


def _build(w):
    """Build + compile the per-core SPMD program with weights baked in."""
    nc = bacc.Bacc("TRN2", target_bir_lowering=False, debug=False)
    x = nc.dram_tensor("x", [S_PER_CORE * C * 128, T], F32, kind="ExternalInput")
    clswt = nc.dram_tensor("clswt", [128, 24], F32, kind="ExternalInput")
    out = nc.dram_tensor("out", [8, 3], F32, kind="ExternalOutput")

    w2, w4, w6, w8 = w["w2"], w["w4"], w["w6"], w["w8"]
    stt = nc.vector.scalar_tensor_tensor

    # S1 factoring: y0 = kf*y0' + b0 with y0' = scaled_parity*s1s + other.
    w00, w01 = w["w00"], w["w01"]
    if abs(w00) >= abs(w01):
        kf = w00 if w00 != 0.0 else 1.0
        s1s, scale_even = w01 / kf, False   # y0' = xe + s*xo -> in0 = xo
    else:
        kf = w01
        s1s, scale_even = w00 / kf, True    # y0' = s*xe + xo -> in0 = xe
    pool3 = Alu.max if kf > 0 else Alu.min

    with tile.TileContext(nc) as tc:
        with (
            tc.tile_pool(name="xpool", bufs=4) as xpool,
            tc.tile_pool(name="wk", bufs=2) as wk,
            tc.tile_pool(name="const", bufs=1) as const,
            tc.tile_pool(name="psum", bufs=1, space="PSUM") as psum,
        ):
            clsw = const.tile([128, 24], F32)
            featmat = const.tile([128, 64], F32)

            # y0' phase planes: [p, 3 planes, C groups, 200]
            y0 = const.tile([128, 3 * C * 200], F32)
            y0b = y0[:]
            y0v = y0[:].rearrange("p (i c t) -> p i c t", i=3, c=C, t=200)
            # 3-way pool staging (interleaved [p, C, 200])
            t01 = const.tile([128, C * 200], F32)
            mtl = const.tile([128, C * 200], F32)
            t01b, mb = t01[:], mtl[:]

            # per-partition bias vector for the S2b Relu activation
            b0c = const.tile([128, 1], F32)
            nc.gpsimd.memset(b0c[:], w["b0"])

            # persistent staging for the batched small stages
            r3all = const.tile([128, S_PER_CORE * C * 50], F32)
            y4all = const.tile([128, S_PER_CORE * C * 25], F32)
            r5all = const.tile([128, S_PER_CORE * C * 12], F32)
            y6all = const.tile([128, S_PER_CORE * C * 6], F32)
            r7all = const.tile([128, S_PER_CORE * C * 3], F32)
            fball = const.tile([128, S_PER_CORE * C], F32)
            r3v = r3all[:].rearrange("p (s c t) -> p s c t", s=S_PER_CORE, c=C)
            y4v = y4all[:].rearrange("p (s c t) -> p s c t", s=S_PER_CORE, c=C)
            r5v = r5all[:].rearrange("p (s c t) -> p s c t", s=S_PER_CORE, c=C)
            y6v = y6all[:].rearrange("p (s c t) -> p s c t", s=S_PER_CORE, c=C)
            r7v = r7all[:].rearrange("p (s c t) -> p s c t", s=S_PER_CORE, c=C)
            fbv = fball[:].rearrange("p (s c) -> p s c", s=S_PER_CORE)
            fmv = featmat[:].rearrange("p (s c) -> p s c", s=S_PER_CORE)

            x4 = x[:].rearrange("(s c p) t -> s p c t", s=S_PER_CORE, c=C, p=128)

            def emit_pool_relu(eob, c0, cn):
                """S2 maxpool3 + S2b Relu-affine into padded E/O planes,
                for groups [c0, c0+cn)."""
                t01s = _cap(t01b, c0 * 200, [(1, cn * 200)])
                nc.vector.tensor_tensor(
                    t01s, _cap(y0b, c0 * 200, [(1, cn * 200)]),
                    _cap(y0b, 1600 + c0 * 200, [(1, cn * 200)]), pool3)
                ms = _cap(mb, c0 * 200, [(1, cn * 200)])
                nc.vector.tensor_tensor(
                    ms, t01s, _cap(y0b, 3200 + c0 * 200, [(1, cn * 200)]), pool3)
                nc.gpsimd.memset(_cap(eob, c0 * 101 + 100, [(101, cn)]), 0.0)
                nc.gpsimd.memset(_cap(eob, c0 * 101 + 808, [(101, cn)]), 0.0)
                nc.scalar.activation(
                    _cap(eob, c0 * 101, [(101, cn), (1, 100)]),
                    _cap(mb, c0 * 200, [(200, cn), (2, 100)]),
                    Act.Relu, bias=b0c[:], scale=kf)
                nc.scalar.activation(
                    _cap(eob, c0 * 101 + 809, [(101, cn), (1, 100)]),
                    _cap(mb, c0 * 200 + 1, [(200, cn), (2, 100)]),
                    Act.Relu, bias=b0c[:], scale=kf)

            def emit_conv2_act(eob, y2b, c0, cn):
                """ScalarE base tap of conv2 for groups [c0, c0+cn)."""
                nc.scalar.activation(
                    _cap(y2b, c0 * 100, [(100, cn), (1, 100)]),
                    _cap(eob, c0 * 101, [(101, cn), (1, 100)]),
                    Act.Copy, bias=w["b2"], scale=w2[1])

            def emit_conv2_pool(s, eob, y2b, c0, cn):
                """DVE taps of conv2 + S4 maxpool2+relu for groups [c0, c0+cn)."""
                ydst = _cap(y2b, c0 * 100, [(100, cn), (1, 100)])
                for off, wt in ((809, w2[2]), (808, w2[0]), (1, w2[3])):
                    stt(ydst, _cap(eob, c0 * 101 + off, [(101, cn), (1, 100)]),
                        wt, ydst, Alu.mult, Alu.add)
                stt(_cap(r3all[:], s * 400 + c0 * 50, [(1, cn * 50)]),
                    _cap(y2b, c0 * 100, [(100, cn), (2, 50)]), 0.0,
                    _cap(y2b, c0 * 100 + 1, [(100, cn), (2, 50)]),
                    Alu.max, Alu.max)

            def tail_batch(lo, hi, c0=0, cn=C):
                """S5..S9 for supertiles [lo, hi) x groups [c0, c0+cn)."""
                sl = slice(lo, hi)
                cs = slice(c0, c0 + cn)
                R3 = r3v[:, sl, cs]
                Y4 = y4v[:, sl, cs]
                nc.scalar.activation(Y4, R3[:, :, :, 0:50:2], Act.Copy,
                                     bias=w["b4"], scale=w4[1])
                stt(Y4, R3[:, :, :, 1:50:2], w4[2], Y4, Alu.mult, Alu.add)
                stt(Y4[:, :, :, 1:25], R3[:, :, :, 1:48:2], w4[0],
                    Y4[:, :, :, 1:25], Alu.mult, Alu.add)
                stt(Y4[:, :, :, 0:24], R3[:, :, :, 2:49:2], w4[3],
                    Y4[:, :, :, 0:24], Alu.mult, Alu.add)
                R5 = r5v[:, sl, cs]
                stt(R5, Y4[:, :, :, 0:24:2], 0.0, Y4[:, :, :, 1:25:2],
                    Alu.max, Alu.max)
                Y6 = y6v[:, sl, cs]
                nc.scalar.activation(Y6, R5[:, :, :, 0:12:2], Act.Copy,
                                     bias=w["b6"], scale=w6[1])
                stt(Y6, R5[:, :, :, 1:12:2], w6[2], Y6, Alu.mult, Alu.add)
                stt(Y6[:, :, :, 1:6], R5[:, :, :, 1:10:2], w6[0],
                    Y6[:, :, :, 1:6], Alu.mult, Alu.add)
                stt(Y6[:, :, :, 0:5], R5[:, :, :, 2:11:2], w6[3],
                    Y6[:, :, :, 0:5], Alu.mult, Alu.add)
                R7 = r7v[:, sl, cs]
                stt(R7, Y6[:, :, :, 0:6:2], 0.0, Y6[:, :, :, 1:6:2],
                    Alu.max, Alu.max)
                FB = fbv[:, sl, cs]
                nc.scalar.activation(FB, R7[:, :, :, 0], Act.Copy,
                                     bias=w["b8"], scale=w8[0])
                stt(FB, R7[:, :, :, 1], w8[1], FB, Alu.mult, Alu.add)
                stt(fmv[:, sl, cs], R7[:, :, :, 2], w8[2], FB, Alu.mult, Alu.add)

            def emit_s1(xh, h):
                """conv0 k2 s2 for half-supertile h -> y0' phase planes.
                dst planes [(1600,3),(1,800)]; srcs [(2,3),(6,800)] (group
                stride 1200 = 6*200 merges with t)."""
                xb = xh[:]
                se = _cap(xb, 0, [(2, 3), (6, 800)])
                so = _cap(xb, 1, [(2, 3), (6, 800)])
                dst = _cap(y0b, h * 800, [(1600, 3), (1, 800)])
                if scale_even:
                    stt(dst, se, s1s, so, Alu.mult, Alu.add)
                else:
                    stt(dst, so, s1s, se, Alu.mult, Alu.add)

            H = C // 2
            prev = None   # (s, eob, y2b) with Vec-side S3/S4 still pending
            for s in range(S_PER_CORE):
                xts = []
                for h in range(2):
                    xh = xpool.tile([128, H * T], F32)
                    nc.sync.dma_start(
                        xh[:].rearrange("p (c t) -> p c t", c=H),
                        x4[s][:, h * H:(h + 1) * H])
                    xts.append(xh)

                if prev is not None:
                    ps, peo, py2 = prev
                    emit_conv2_pool(ps, peo, py2, 0, C)
                    if ps == 3:
                        tail_batch(0, 4)
                    elif ps == 5:
                        tail_batch(4, 6)
                    elif ps == 6:
                        tail_batch(6, 7)

                eo = wk.tile([128, 2 * C * 101], F32)
                y2 = wk.tile([128, C * 100], F32)
                eob, y2b = eo[:], y2[:]
                if s < S_PER_CORE - 1:
                    emit_s1(xts[0], 0)
                    emit_s1(xts[1], 1)
                    emit_pool_relu(eob, 0, C)
                    emit_conv2_act(eob, y2b, 0, C)
                    prev = (s, eob, y2b)
                else:
                    # drain supertile: process halves immediately so only
                    # half a supertile of compute trails the last DMA byte
                    for h in range(2):
                        emit_s1(xts[h], h)
                        emit_pool_relu(eob, h * H, H)
                        emit_conv2_act(eob, y2b, h * H, H)
                        emit_conv2_pool(s, eob, y2b, h * H, H)
                        tail_batch(7, 8, h * H, H)

            # classifier weights load late so the first x DMA issues first
            nc.sync.dma_start(clsw[:], clswt[:])
            # classifier: logits[s, cls] = sum_c featmat[:, c::8].T @ clsw
            lg = psum.tile([8, 3], F32)
            for c in range(C):
                nc.tensor.matmul(lg[:], featmat[:, c::8], clsw[:, c * 3:(c + 1) * 3],
                                 start=(c == 0), stop=(c == C - 1))
            if any(v != 0.0 for v in w["cls_b"]):
                lgs = const.tile([8, 3], F32)
                nc.vector.tensor_copy(lgs[:], lg[:])
                for cls in range(3):
                    if w["cls_b"][cls] != 0.0:
                        nc.vector.tensor_scalar_add(lgs[:, cls:cls + 1],
                                                    lgs[:, cls:cls + 1],
                                                    w["cls_b"][cls])
                lsrc = lgs[:]
            else:
                lsrc = lg[:]   # zero bias: reduce + Exp read PSUM directly
            # softmax (max-subtracted, like jax.nn.softmax)
            nmx = const.tile([8, 1], F32)
            nc.vector.tensor_reduce(nmx[:], lsrc, mybir.AxisListType.X, Alu.max,
                                    negate=True)
            ex = const.tile([8, 3], F32)
            smv = const.tile([8, 1], F32)
            nc.scalar.activation(ex[:], lsrc, Act.Exp, bias=nmx[:], scale=1.0,
                                 accum_out=smv[:])
            ri = const.tile([8, 1], F32)
            nc.vector.reciprocal(ri[:], smv[:])
            pr = const.tile([8, 3], F32)
            nc.vector.tensor_scalar(pr[:], ex[:], ri[:], None, Alu.mult)
            nc.sync.dma_start(out[:], pr[:])

    nc.compile()
    return nc


def _extract_weights(inputs):
    f = lambda a: [float(v) for v in np.asarray(a).reshape(-1)]
    return dict(
        w00=f(inputs["c0_w"])[0], w01=f(inputs["c0_w"])[1], b0=f(inputs["c0_b"])[0],
        w2=f(inputs["c2_w"]), b2=f(inputs["c2_b"])[0],
        w4=f(inputs["c4_w"]), b4=f(inputs["c4_b"])[0],
        w6=f(inputs["c6_w"]), b6=f(inputs["c6_b"])[0],
        w8=f(inputs["c8_w"]), b8=f(inputs["c8_b"])[0],
        cls_b=f(inputs["cls_b"]),
    )


def _run(inputs, trace=False, trace_kwargs=None):
    w = _extract_weights(inputs)
    key = tuple(np.asarray(
        [w["w00"], w["w01"], w["b0"]] + w["w2"] + [w["b2"]] + w["w4"] + [w["b4"]]
        + w["w6"] + [w["b6"]] + w["w8"] + [w["b8"]] + w["cls_b"], np.float64
    ).tobytes())
    if key not in _CACHE:
        _CACHE[key] = _build(w)
    nc = _CACHE[key]

    x = np.ascontiguousarray(np.asarray(inputs["x"], dtype=np.float32))
    xf = x.reshape(BS * NN, T)
    cls_w = np.asarray(inputs["cls_w"], dtype=np.float32)       # (3, 1024)
    clsT = np.empty((128, 24), np.float32)
    for c in range(C):
        clsT[:, c * 3:(c + 1) * 3] = cls_w[:, c * 128:(c + 1) * 128].T

    rows_per_core = BS * NN // N_CORES
    in_maps = [
        {"x": np.ascontiguousarray(xf[i * rows_per_core:(i + 1) * rows_per_core]),
         "clswt": clsT}
        for i in range(N_CORES)
    ]
    res = run_bass_kernel_spmd(nc, in_maps, list(range(N_CORES)), trace=trace,
                               **(trace_kwargs or {}))
    out = np.concatenate([np.asarray(res.results[i]["out"]) for i in range(N_CORES)],
                         axis=0).astype(np.float32)
    return out, res


def kernel(**inputs):
    out, _ = _run(inputs, trace=False)
    return out


# revision 8
# speedup vs baseline: 1.1168x; 1.1168x over previous
"""Trainium2 Bass kernel for nn_DeepTimeGraphNet (per-row conv/pool pyramid + classifier).

Contract: kernel(**inputs) takes the FULL unsharded inputs (keys as in
setup_inputs()) and returns the FULL (64, 3) softmax output.

Sharding: pure data parallel over batch. Core i handles batch rows
[8i, 8i+8) = 8192 (batch, node) rows of length 1200, processed as 8
supertiles of 1024 rows = 128 SBUF partitions x 8 column groups.

v3 design, from measured engine rates (DVE stt/TT and ScalarE ACT are all
~0.96 el/ns on fp32 regardless of stride/dtype, EXCEPT contiguous fp16
tensor_tensor which hits ~1.98 el/ns; GpSimd cannot run tensor ops):

- S1 (conv0 k2 s2): ScalarE computes both taps as separate fp16 buffers
  (ae = w00*x_even + b0, ao = w01*x_odd) via ACTIVATE at ~1 el/ns, and
  DVE combines them with one contiguous fp16 TT add at ~2 el/ns. This
  moves ~half of conv0 off the (otherwise oversubscribed) DVE.
- S2 (maxpool3+relu): fp16 TT max + fp16 stt max-max (relu fused via the
  scalar-0 slot), stride-3 reads.
- S3 (conv2 k4 s2 p1): ScalarE base tap (bias), 3 DVE fp16 stt taps.
- S4 (maxpool2+relu): one stt, fp16 srcs -> fp32 r3 staging.
- 2-deep software pipeline: iteration s emits DVE taps/S4 of s-2 and
  DVE add/pools of s-1 while ScalarE loads supertile s, so neither
  engine ever waits on the other's round trip.
- Tail (conv4..conv8, 50->1, fp32) batched over supertile groups
  [0,4) [4,6) [6,8) placed in DMA windows; PE matmul classifier +
  exact softmax.

Per-supertile engine budget: DVE ~11.5us, ScalarE ~11.4us, both under
the 12.5us DMA window (39.3 MB/core over 16 DMA engines x ~24.6 B/ns
~= 100us floor), so the stream is DMA-paced.
"""
import os
import sys

for _p in ("/root/.axon_site/_ro/trn_rl_repo", "/opt/trn_rl_repo"):
    if os.path.isdir(_p) and _p not in sys.path:
        sys.path.insert(0, _p)

import numpy as np  # noqa: E402

import concourse.bacc as bacc  # noqa: E402
import concourse.tile as tile  # noqa: E402
from concourse import mybir  # noqa: E402
from concourse.bass_utils import run_bass_kernel_spmd  # noqa: E402

F32 = mybir.dt.float32
F16 = mybir.dt.float16
Alu = mybir.AluOpType
Act = mybir.ActivationFunctionType

BS, NN, T = 64, 1024, 1200
N_CORES = 8
S_PER_CORE = 8          # supertiles per core; each = 1024 rows (one batch row)
C = 8                   # column groups per supertile (128 rows each)

_CACHE = {}


def _build(w):
    """Build + compile the per-core SPMD program with weights baked in."""
    nc = bacc.Bacc("TRN2", target_bir_lowering=False, debug=False)
    x = nc.dram_tensor("x", [S_PER_CORE * C * 128, T], F32, kind="ExternalInput")
    clswt = nc.dram_tensor("clswt", [128, 24], F32, kind="ExternalInput")
    out = nc.dram_tensor("out", [8, 3], F32, kind="ExternalOutput")

    w2, w4, w6, w8 = w["w2"], w["w4"], w["w6"], w["w8"]
    stt = nc.vector.scalar_tensor_tensor

    with tile.TileContext(nc) as tc:
        with (
            tc.tile_pool(name="xpool", bufs=4) as xpool,
            tc.tile_pool(name="aeo", bufs=8) as aeo,
            tc.tile_pool(name="wk", bufs=2) as wk,
            tc.tile_pool(name="const", bufs=1) as const,
            tc.tile_pool(name="psum", bufs=1, space="PSUM") as psum,
        ):
            clsw = const.tile([128, 24], F32)
            featmat = const.tile([128, 64], F32)

            y0 = const.tile([128, C * 600], F16)
            y0v = y0[:].rearrange("p (c t) -> p c t", c=C)
            t01 = const.tile([128, C * 200], F16)
            t01v = t01[:].rearrange("p (c t) -> p c t", c=C)

            # persistent staging for the batched small stages (fp32)
            r3all = const.tile([128, S_PER_CORE * C * 50], F32)
            y4all = const.tile([128, S_PER_CORE * C * 25], F32)
            r5all = const.tile([128, S_PER_CORE * C * 12], F32)
            y6all = const.tile([128, S_PER_CORE * C * 6], F32)
            r7all = const.tile([128, S_PER_CORE * C * 3], F32)
            fball = const.tile([128, S_PER_CORE * C], F32)
            r3v = r3all[:].rearrange("p (s c t) -> p s c t", s=S_PER_CORE, c=C)
            y4v = y4all[:].rearrange("p (s c t) -> p s c t", s=S_PER_CORE, c=C)
            r5v = r5all[:].rearrange("p (s c t) -> p s c t", s=S_PER_CORE, c=C)
            y6v = y6all[:].rearrange("p (s c t) -> p s c t", s=S_PER_CORE, c=C)
            r7v = r7all[:].rearrange("p (s c t) -> p s c t", s=S_PER_CORE, c=C)
            fbv = fball[:].rearrange("p (s c) -> p s c", s=S_PER_CORE)
            fmv = featmat[:].rearrange("p (s c) -> p s c", s=S_PER_CORE)

            x4 = x[:].rearrange("(s c p) t -> s p c t", s=S_PER_CORE, c=C, p=128)

            st = {}   # per-supertile handles

            def sca_ae_ao(s):
                """S1 taps on ScalarE: ae = w00*x_even + b0, ao = w01*x_odd
                (fp16), per half-supertile."""
                d = st[s]
                d["ae"], d["ao"] = [], []
                for h in range(2):
                    xb = d["x"][h][:]
                    ae = aeo.tile([128, 2400], F16)
                    ao = aeo.tile([128, 2400], F16)
                    nc.scalar.activation(ae[:], xb[:, 0:4800:2], Act.Copy,
                                         bias=w["b0"], scale=w["w00"])
                    nc.scalar.activation(ao[:], xb[:, 1:4800:2], Act.Copy,
                                         bias=0.0, scale=w["w01"])
                    d["ae"].append(ae)
                    d["ao"].append(ao)

            def vec_add_pools(s):
                """DVE: y0 = ae + ao (contiguous fp16 TT, 2x rate), then
                maxpool3 + fused relu -> r1 (fp16)."""
                d = st[s]
                for h in range(2):
                    nc.vector.tensor_tensor(y0[:, h * 2400:(h + 1) * 2400],
                                            d["ae"][h][:], d["ao"][h][:],
                                            Alu.add)
                nc.vector.tensor_tensor(t01v, y0v[:, :, 0:600:3],
                                        y0v[:, :, 1:600:3], Alu.max)
                r1 = wk.tile([128, C * 200], F16)
                d["r1"] = r1
                stt(r1[:].rearrange("p (c t) -> p c t", c=C), t01v, 0.0,
                    y0v[:, :, 2:600:3], Alu.max, Alu.max)

            def sca_s3base(s):
                """ScalarE base tap of conv2: y2 = w2[1]*r1_even + b2."""
                d = st[s]
                y2 = wk.tile([128, C * 100], F16)
                d["y2"] = y2
                r1v = d["r1"][:].rearrange("p (c t) -> p c t", c=C)
                nc.scalar.activation(y2[:].rearrange("p (c t) -> p c t", c=C),
                                     r1v[:, :, 0:200:2], Act.Copy,
                                     bias=w["b2"], scale=w2[1])

            def vec_taps_s4(s):
                """DVE taps of conv2 + S4 maxpool2+relu -> fp32 r3."""
                d = st[s]
                r1v = d["r1"][:].rearrange("p (c t) -> p c t", c=C)
                y2v = d["y2"][:].rearrange("p (c t) -> p c t", c=C)
                stt(y2v, r1v[:, :, 1:200:2], w2[2], y2v, Alu.mult, Alu.add)
                stt(y2v[:, :, 1:100], r1v[:, :, 1:198:2], w2[0],
                    y2v[:, :, 1:100], Alu.mult, Alu.add)
                stt(y2v[:, :, 0:99], r1v[:, :, 2:199:2], w2[3],
                    y2v[:, :, 0:99], Alu.mult, Alu.add)
                stt(r3v[:, s], y2v[:, :, 0:100:2], 0.0, y2v[:, :, 1:100:2],
                    Alu.max, Alu.max)

            def tail_batch(lo, hi):
                """S5..S9 batched over supertiles [lo, hi) (fp32)."""
                sl = slice(lo, hi)
                R3 = r3v[:, sl]
                Y4 = y4v[:, sl]
                nc.scalar.activation(Y4, R3[:, :, :, 0:50:2], Act.Copy,
                                     bias=w["b4"], scale=w4[1])
                stt(Y4, R3[:, :, :, 1:50:2], w4[2], Y4, Alu.mult, Alu.add)
                stt(Y4[:, :, :, 1:25], R3[:, :, :, 1:48:2], w4[0],
                    Y4[:, :, :, 1:25], Alu.mult, Alu.add)
                stt(Y4[:, :, :, 0:24], R3[:, :, :, 2:49:2], w4[3],
                    Y4[:, :, :, 0:24], Alu.mult, Alu.add)
                R5 = r5v[:, sl]
                stt(R5, Y4[:, :, :, 0:24:2], 0.0, Y4[:, :, :, 1:25:2],
                    Alu.max, Alu.max)
                Y6 = y6v[:, sl]
                nc.scalar.activation(Y6, R5[:, :, :, 0:12:2], Act.Copy,
                                     bias=w["b6"], scale=w6[1])
                stt(Y6, R5[:, :, :, 1:12:2], w6[2], Y6, Alu.mult, Alu.add)
                stt(Y6[:, :, :, 1:6], R5[:, :, :, 1:10:2], w6[0],
                    Y6[:, :, :, 1:6], Alu.mult, Alu.add)
                stt(Y6[:, :, :, 0:5], R5[:, :, :, 2:11:2], w6[3],
                    Y6[:, :, :, 0:5], Alu.mult, Alu.add)
                R7 = r7v[:, sl]
                stt(R7, Y6[:, :, :, 0:6:2], 0.0, Y6[:, :, :, 1:6:2],
                    Alu.max, Alu.max)
                FB = fbv[:, sl]
                nc.scalar.activation(FB, R7[:, :, :, 0], Act.Copy,
                                     bias=w["b8"], scale=w8[0])
                stt(FB, R7[:, :, :, 1], w8[1], FB, Alu.mult, Alu.add)
                stt(fmv[:, sl], R7[:, :, :, 2], w8[2], FB, Alu.mult, Alu.add)

            H = C // 2
            for s in range(S_PER_CORE):
                st[s] = {"x": []}
                for h in range(2):
                    xh = xpool.tile([128, H * T], F32)
                    nc.sync.dma_start(
                        xh[:].rearrange("p (c t) -> p c t", c=H),
                        x4[s][:, h * H:(h + 1) * H])
                    st[s]["x"].append(xh)

                if s >= 2:
                    vec_taps_s4(s - 2)
                    st.pop(s - 2)
                if s == 5:
                    tail_batch(0, 4)
                elif s == 7:
                    tail_batch(4, 6)
                if s >= 1:
                    vec_add_pools(s - 1)
                sca_ae_ao(s)
                if s >= 1:
                    sca_s3base(s - 1)

            # drain: taps/S4 of s=6 + its tail chew on DVE while the last
            # DMA streams; then the short s=7 chain.
            vec_taps_s4(6)
            tail_batch(6, 7)
            vec_add_pools(7)
            sca_s3base(7)
            vec_taps_s4(7)
            tail_batch(7, 8)

            # classifier weights load late so the first x DMA issues first
            nc.sync.dma_start(clsw[:], clswt[:])
            # classifier: logits[s, cls] = sum_c featmat[:, c::8].T @ clsw
            lg = psum.tile([8, 3], F32)
            for c in range(C):
                nc.tensor.matmul(lg[:], featmat[:, c::8], clsw[:, c * 3:(c + 1) * 3],
                                 start=(c == 0), stop=(c == C - 1))
            if any(v != 0.0 for v in w["cls_b"]):
                lgs = const.tile([8, 3], F32)
                nc.vector.tensor_copy(lgs[:], lg[:])
                for cls in range(3):
                    if w["cls_b"][cls] != 0.0:
                        nc.vector.tensor_scalar_add(lgs[:, cls:cls + 1],
                                                    lgs[:, cls:cls + 1],
                                                    w["cls_b"][cls])
                lsrc = lgs[:]
            else:
                lsrc = lg[:]   # zero bias: reduce + Exp read PSUM directly
            # softmax (max-subtracted, like jax.nn.softmax)
            nmx = const.tile([8, 1], F32)
            nc.vector.tensor_reduce(nmx[:], lsrc, mybir.AxisListType.X, Alu.max,
                                    negate=True)
            ex = const.tile([8, 3], F32)
            smv = const.tile([8, 1], F32)
            nc.scalar.activation(ex[:], lsrc, Act.Exp, bias=nmx[:], scale=1.0,
                                 accum_out=smv[:])
            ri = const.tile([8, 1], F32)
            nc.vector.reciprocal(ri[:], smv[:])
            pr = const.tile([8, 3], F32)
            nc.vector.tensor_scalar(pr[:], ex[:], ri[:], None, Alu.mult)
            nc.sync.dma_start(out[:], pr[:])

    nc.compile()
    return nc


def _extract_weights(inputs):
    f = lambda a: [float(v) for v in np.asarray(a).reshape(-1)]
    return dict(
        w00=f(inputs["c0_w"])[0], w01=f(inputs["c0_w"])[1], b0=f(inputs["c0_b"])[0],
        w2=f(inputs["c2_w"]), b2=f(inputs["c2_b"])[0],
        w4=f(inputs["c4_w"]), b4=f(inputs["c4_b"])[0],
        w6=f(inputs["c6_w"]), b6=f(inputs["c6_b"])[0],
        w8=f(inputs["c8_w"]), b8=f(inputs["c8_b"])[0],
        cls_b=f(inputs["cls_b"]),
    )


def _run(inputs, trace=False, trace_kwargs=None):
    w = _extract_weights(inputs)
    key = tuple(np.asarray(
        [w["w00"], w["w01"], w["b0"]] + w["w2"] + [w["b2"]] + w["w4"] + [w["b4"]]
        + w["w6"] + [w["b6"]] + w["w8"] + [w["b8"]] + w["cls_b"], np.float64
    ).tobytes())
    if key not in _CACHE:
        _CACHE[key] = _build(w)
    nc = _CACHE[key]

    x = np.ascontiguousarray(np.asarray(inputs["x"], dtype=np.float32))
    xf = x.reshape(BS * NN, T)
    cls_w = np.asarray(inputs["cls_w"], dtype=np.float32)       # (3, 1024)
    clsT = np.empty((128, 24), np.float32)
    for c in range(C):
        clsT[:, c * 3:(c + 1) * 3] = cls_w[:, c * 128:(c + 1) * 128].T

    rows_per_core = BS * NN // N_CORES
    in_maps = [
        {"x": np.ascontiguousarray(xf[i * rows_per_core:(i + 1) * rows_per_core]),
         "clswt": clsT}
        for i in range(N_CORES)
    ]
    res = run_bass_kernel_spmd(nc, in_maps, list(range(N_CORES)), trace=trace,
                               **(trace_kwargs or {}))
    out = np.concatenate([np.asarray(res.results[i]["out"]) for i in range(N_CORES)],
                         axis=0).astype(np.float32)
    return out, res


def kernel(**inputs):
    out, _ = _run(inputs, trace=False)
    return out


# revision 11
# speedup vs baseline: 1.1444x; 1.0248x over previous
"""Trainium2 Bass kernel for nn_DeepTimeGraphNet (per-row conv/pool pyramid + classifier).

Contract: kernel(**inputs) takes the FULL unsharded inputs (keys as in
setup_inputs()) and returns the FULL (64, 3) softmax output.

Sharding: pure data parallel over batch. Core i handles batch rows
[8i, 8i+8) = 8192 (batch, node) rows of length 1200, processed as 8
supertiles of 1024 rows = 128 SBUF partitions x 8 column groups.

v3 design, from measured engine rates (DVE stt/TT and ScalarE ACT are all
~0.96 el/ns on fp32 regardless of stride/dtype, EXCEPT contiguous fp16
tensor_tensor which hits ~1.98 el/ns; GpSimd cannot run tensor ops):

- S1 (conv0 k2 s2): ScalarE computes both taps as separate fp16 buffers
  (ae = w00*x_even + b0, ao = w01*x_odd) via ACTIVATE at ~1 el/ns, and
  DVE combines them with one contiguous fp16 TT add at ~2 el/ns. This
  moves ~half of conv0 off the (otherwise oversubscribed) DVE.
- S2 (maxpool3+relu): fp16 TT max + fp16 stt max-max (relu fused via the
  scalar-0 slot), stride-3 reads.
- S3 (conv2 k4 s2 p1): ScalarE base tap (bias), 3 DVE fp16 stt taps.
- S4 (maxpool2+relu): one stt, fp16 srcs -> fp32 r3 staging.
- 2-deep software pipeline: iteration s emits DVE taps/S4 of s-2 and
  DVE add/pools of s-1 while ScalarE loads supertile s, so neither
  engine ever waits on the other's round trip.
- Tail (conv4..conv8, 50->1, fp32) batched over supertile groups
  [0,4) [4,6) [6,8) placed in DMA windows; PE matmul classifier +
  exact softmax.

Per-supertile engine budget: DVE ~11.5us, ScalarE ~11.4us, both under
the 12.5us DMA window (39.3 MB/core over 16 DMA engines x ~24.6 B/ns
~= 100us floor), so the stream is DMA-paced.
"""
import os
import sys

for _p in ("/root/.axon_site/_ro/trn_rl_repo", "/opt/trn_rl_repo"):
    if os.path.isdir(_p) and _p not in sys.path:
        sys.path.insert(0, _p)

import numpy as np  # noqa: E402

import concourse.bacc as bacc  # noqa: E402
import concourse.tile as tile  # noqa: E402
from concourse import mybir  # noqa: E402
from concourse.bass_utils import run_bass_kernel_spmd  # noqa: E402

F32 = mybir.dt.float32
F16 = mybir.dt.float16
Alu = mybir.AluOpType
Act = mybir.ActivationFunctionType

BS, NN, T = 64, 1024, 1200
N_CORES = 8
S_PER_CORE = 8          # supertiles per core; each = 1024 rows (one batch row)
C = 8                   # column groups per supertile (128 rows each)

_CACHE = {}


def _build(w):
    """Build + compile the per-core SPMD program with weights baked in."""
    nc = bacc.Bacc("TRN2", target_bir_lowering=False, debug=False)
    x = nc.dram_tensor("x", [S_PER_CORE * C * 128, T], F32, kind="ExternalInput")
    clswt = nc.dram_tensor("clswt", [128, 24], F32, kind="ExternalInput")
    out = nc.dram_tensor("out", [8, 3], F32, kind="ExternalOutput")

    w2, w4, w6, w8 = w["w2"], w["w4"], w["w6"], w["w8"]
    stt = nc.vector.scalar_tensor_tensor

    with tile.TileContext(nc) as tc:
        with (
            tc.tile_pool(name="xpool", bufs=8) as xpool,
            tc.tile_pool(name="aeo", bufs=16) as aeo,
            tc.tile_pool(name="wk", bufs=2) as wk,
            tc.tile_pool(name="const", bufs=1) as const,
            tc.tile_pool(name="psum", bufs=1, space="PSUM") as psum,
        ):
            clsw = const.tile([128, 24], F32)
            featmat = const.tile([128, 64], F32)

            y0 = const.tile([128, C * 600], F16)
            y0v = y0[:].rearrange("p (c t) -> p c t", c=C)
            t01 = const.tile([128, C * 200], F16)
            t01v = t01[:].rearrange("p (c t) -> p c t", c=C)

            # persistent staging for the batched small stages (fp32)
            r3all = const.tile([128, S_PER_CORE * C * 50], F32)
            y4all = const.tile([128, S_PER_CORE * C * 25], F32)
            r5all = const.tile([128, S_PER_CORE * C * 12], F32)
            y6all = const.tile([128, S_PER_CORE * C * 6], F32)
            r7all = const.tile([128, S_PER_CORE * C * 3], F32)
            fball = const.tile([128, S_PER_CORE * C], F32)
            r3v = r3all[:].rearrange("p (s c t) -> p s c t", s=S_PER_CORE, c=C)
            y4v = y4all[:].rearrange("p (s c t) -> p s c t", s=S_PER_CORE, c=C)
            r5v = r5all[:].rearrange("p (s c t) -> p s c t", s=S_PER_CORE, c=C)
            y6v = y6all[:].rearrange("p (s c t) -> p s c t", s=S_PER_CORE, c=C)
            r7v = r7all[:].rearrange("p (s c t) -> p s c t", s=S_PER_CORE, c=C)
            fbv = fball[:].rearrange("p (s c) -> p s c", s=S_PER_CORE)
            fmv = featmat[:].rearrange("p (s c) -> p s c", s=S_PER_CORE)

            x4 = x[:].rearrange("(s c p) t -> s p c t", s=S_PER_CORE, c=C, p=128)

            st = {}   # per-supertile handles

            def sca_ae_ao(s):
                """S1 taps on ScalarE: ae = w00*x_even + b0, ao = w01*x_odd
                (fp16), per quarter-supertile (fine slot recycling keeps the
                DMA queues fed and completions arriving every ~3us)."""
                d = st[s]
                d["ae"], d["ao"] = [], []
                for q in range(4):
                    xb = d["x"][q][:]
                    ae = aeo.tile([128, 1200], F16)
                    ao = aeo.tile([128, 1200], F16)
                    nc.scalar.activation(ae[:], xb[:, 0:2400:2], Act.Copy,
                                         bias=w["b0"], scale=w["w00"])
                    nc.scalar.activation(ao[:], xb[:, 1:2400:2], Act.Copy,
                                         bias=0.0, scale=w["w01"])
                    d["ae"].append(ae)
                    d["ao"].append(ao)

            def vec_add_pools(s):
                """DVE: y0 = ae + ao (contiguous fp16 TT, 2x rate), then
                maxpool3 + fused relu -> r1 (fp16)."""
                d = st[s]
                for q in range(4):
                    nc.vector.tensor_tensor(y0[:, q * 1200:(q + 1) * 1200],
                                            d["ae"][q][:], d["ao"][q][:],
                                            Alu.add)
                nc.vector.tensor_tensor(t01v, y0v[:, :, 0:600:3],
                                        y0v[:, :, 1:600:3], Alu.max)
                r1 = wk.tile([128, C * 200], F16)
                d["r1"] = r1
                stt(r1[:].rearrange("p (c t) -> p c t", c=C), t01v, 0.0,
                    y0v[:, :, 2:600:3], Alu.max, Alu.max)

            def sca_s3base(s):
                """ScalarE base tap of conv2: y2 = w2[1]*r1_even + b2."""
                d = st[s]
                y2 = wk.tile([128, C * 100], F16)
                d["y2"] = y2
                r1v = d["r1"][:].rearrange("p (c t) -> p c t", c=C)
                nc.scalar.activation(y2[:].rearrange("p (c t) -> p c t", c=C),
                                     r1v[:, :, 0:200:2], Act.Copy,
                                     bias=w["b2"], scale=w2[1])

            def vec_taps_s4(s):
                """DVE taps of conv2 + S4 maxpool2+relu -> fp32 r3."""
                d = st[s]
                r1v = d["r1"][:].rearrange("p (c t) -> p c t", c=C)
                y2v = d["y2"][:].rearrange("p (c t) -> p c t", c=C)
                stt(y2v, r1v[:, :, 1:200:2], w2[2], y2v, Alu.mult, Alu.add)
                stt(y2v[:, :, 1:100], r1v[:, :, 1:198:2], w2[0],
                    y2v[:, :, 1:100], Alu.mult, Alu.add)
                stt(y2v[:, :, 0:99], r1v[:, :, 2:199:2], w2[3],
                    y2v[:, :, 0:99], Alu.mult, Alu.add)
                stt(r3v[:, s], y2v[:, :, 0:100:2], 0.0, y2v[:, :, 1:100:2],
                    Alu.max, Alu.max)

            def tail_batch(lo, hi):
                """S5..S9 batched over supertiles [lo, hi) (fp32)."""
                sl = slice(lo, hi)
                R3 = r3v[:, sl]
                Y4 = y4v[:, sl]
                nc.scalar.activation(Y4, R3[:, :, :, 0:50:2], Act.Copy,
                                     bias=w["b4"], scale=w4[1])
                stt(Y4, R3[:, :, :, 1:50:2], w4[2], Y4, Alu.mult, Alu.add)
                stt(Y4[:, :, :, 1:25], R3[:, :, :, 1:48:2], w4[0],
                    Y4[:, :, :, 1:25], Alu.mult, Alu.add)
                stt(Y4[:, :, :, 0:24], R3[:, :, :, 2:49:2], w4[3],
                    Y4[:, :, :, 0:24], Alu.mult, Alu.add)
                R5 = r5v[:, sl]
                stt(R5, Y4[:, :, :, 0:24:2], 0.0, Y4[:, :, :, 1:25:2],
                    Alu.max, Alu.max)
                Y6 = y6v[:, sl]
                nc.scalar.activation(Y6, R5[:, :, :, 0:12:2], Act.Copy,
                                     bias=w["b6"], scale=w6[1])
                stt(Y6, R5[:, :, :, 1:12:2], w6[2], Y6, Alu.mult, Alu.add)
                stt(Y6[:, :, :, 1:6], R5[:, :, :, 1:10:2], w6[0],
                    Y6[:, :, :, 1:6], Alu.mult, Alu.add)
                stt(Y6[:, :, :, 0:5], R5[:, :, :, 2:11:2], w6[3],
                    Y6[:, :, :, 0:5], Alu.mult, Alu.add)
                R7 = r7v[:, sl]
                stt(R7, Y6[:, :, :, 0:6:2], 0.0, Y6[:, :, :, 1:6:2],
                    Alu.max, Alu.max)
                FB = fbv[:, sl]
                nc.scalar.activation(FB, R7[:, :, :, 0], Act.Copy,
                                     bias=w["b8"], scale=w8[0])
                stt(FB, R7[:, :, :, 1], w8[1], FB, Alu.mult, Alu.add)
                stt(fmv[:, sl], R7[:, :, :, 2], w8[2], FB, Alu.mult, Alu.add)

            for s in range(S_PER_CORE):
                st[s] = {"x": []}
                for q in range(4):
                    xh = xpool.tile([128, 2 * T], F32)
                    nc.sync.dma_start(
                        xh[:].rearrange("p (c t) -> p c t", c=2),
                        x4[s][:, q * 2:(q + 1) * 2])
                    st[s]["x"].append(xh)

                if s >= 2:
                    vec_taps_s4(s - 2)
                    st.pop(s - 2)
                if s == 5:
                    tail_batch(0, 4)
                elif s == 7:
                    tail_batch(4, 6)
                if s >= 1:
                    vec_add_pools(s - 1)
                sca_ae_ao(s)
                if s >= 1:
                    sca_s3base(s - 1)

            # drain: taps/S4 of s=6 + its tail chew on DVE while the last
            # DMA streams; then the short s=7 chain.
            vec_taps_s4(6)
            tail_batch(6, 7)
            vec_add_pools(7)
            sca_s3base(7)
            vec_taps_s4(7)
            tail_batch(7, 8)

            # classifier weights load late so the first x DMA issues first
            nc.sync.dma_start(clsw[:], clswt[:])
            # classifier: logits[s, cls] = sum_c featmat[:, c::8].T @ clsw
            lg = psum.tile([8, 3], F32)
            for c in range(C):
                nc.tensor.matmul(lg[:], featmat[:, c::8], clsw[:, c * 3:(c + 1) * 3],
                                 start=(c == 0), stop=(c == C - 1))
            if any(v != 0.0 for v in w["cls_b"]):
                lgs = const.tile([8, 3], F32)
                nc.vector.tensor_copy(lgs[:], lg[:])
                for cls in range(3):
                    if w["cls_b"][cls] != 0.0:
                        nc.vector.tensor_scalar_add(lgs[:, cls:cls + 1],
                                                    lgs[:, cls:cls + 1],
                                                    w["cls_b"][cls])
                lsrc = lgs[:]
            else:
                lsrc = lg[:]   # zero bias: reduce + Exp read PSUM directly
            # softmax (max-subtracted, like jax.nn.softmax)
            nmx = const.tile([8, 1], F32)
            nc.vector.tensor_reduce(nmx[:], lsrc, mybir.AxisListType.X, Alu.max,
                                    negate=True)
            ex = const.tile([8, 3], F32)
            smv = const.tile([8, 1], F32)
            nc.scalar.activation(ex[:], lsrc, Act.Exp, bias=nmx[:], scale=1.0,
                                 accum_out=smv[:])
            ri = const.tile([8, 1], F32)
            nc.vector.reciprocal(ri[:], smv[:])
            pr = const.tile([8, 3], F32)
            nc.vector.tensor_scalar(pr[:], ex[:], ri[:], None, Alu.mult)
            nc.sync.dma_start(out[:], pr[:])

    nc.compile()
    return nc


def _extract_weights(inputs):
    f = lambda a: [float(v) for v in np.asarray(a).reshape(-1)]
    return dict(
        w00=f(inputs["c0_w"])[0], w01=f(inputs["c0_w"])[1], b0=f(inputs["c0_b"])[0],
        w2=f(inputs["c2_w"]), b2=f(inputs["c2_b"])[0],
        w4=f(inputs["c4_w"]), b4=f(inputs["c4_b"])[0],
        w6=f(inputs["c6_w"]), b6=f(inputs["c6_b"])[0],
        w8=f(inputs["c8_w"]), b8=f(inputs["c8_b"])[0],
        cls_b=f(inputs["cls_b"]),
    )


def _run(inputs, trace=False, trace_kwargs=None):
    w = _extract_weights(inputs)
    key = tuple(np.asarray(
        [w["w00"], w["w01"], w["b0"]] + w["w2"] + [w["b2"]] + w["w4"] + [w["b4"]]
        + w["w6"] + [w["b6"]] + w["w8"] + [w["b8"]] + w["cls_b"], np.float64
    ).tobytes())
    if key not in _CACHE:
        _CACHE[key] = _build(w)
    nc = _CACHE[key]

    x = np.ascontiguousarray(np.asarray(inputs["x"], dtype=np.float32))
    xf = x.reshape(BS * NN, T)
    cls_w = np.asarray(inputs["cls_w"], dtype=np.float32)       # (3, 1024)
    clsT = np.empty((128, 24), np.float32)
    for c in range(C):
        clsT[:, c * 3:(c + 1) * 3] = cls_w[:, c * 128:(c + 1) * 128].T

    rows_per_core = BS * NN // N_CORES
    in_maps = [
        {"x": np.ascontiguousarray(xf[i * rows_per_core:(i + 1) * rows_per_core]),
         "clswt": clsT}
        for i in range(N_CORES)
    ]
    res = run_bass_kernel_spmd(nc, in_maps, list(range(N_CORES)), trace=trace,
                               **(trace_kwargs or {}))
    out = np.concatenate([np.asarray(res.results[i]["out"]) for i in range(N_CORES)],
                         axis=0).astype(np.float32)
    return out, res


def kernel(**inputs):
    out, _ = _run(inputs, trace=False)
    return out
